# revision 2
# baseline (speedup 1.0000x reference)
"""HGTNet Trainium2 kernel v4: dst-ownership + degree-sorted edge layout.

Key ideas:
  - 8-way dst sharding; per-relation kv projections folded on host; kv tables
    AllGather'd in f32 to Shared outputs (measured fast: ~3ms for 307MB/core).
  - Edge phase: per-core dst nodes are HOST-PERMUTED by in-degree (descending)
    so each 128-dst block has near-uniform degree. Edges live at [dst-partition,
    slot-column]: dst p's edges occupy row p. This removes ALL one-hot
    machinery: q selection is a free-axis broadcast, segment-sum is a strided
    row reduce, only 1 PE transpose per block. Padding overhead ~2-4%.
  - The permutation is absorbed into the storage order of h/x/q/kv/agg per
    core (host permutes inputs, unpermutes logits) -- zero device cost.
  - rev pass runs before pap pass so its edge work overlaps the big pap AG.
"""
import sys, os
sys.path.insert(0, '/opt/trn_rl_repo')
import math
import numpy as np

import concourse.bass as bass
import concourse.bacc as bacc
import concourse.mybir as mybir
import concourse.tile as tile
from concourse.bass_utils import run_bass_kernel_spmd

P = 128
D = 128
H = 4
DH = 32
L = 2
OUT = 40
NN = 100000
E = 400000
C = 8
SCALE = 1.0 / math.sqrt(DH)
MASKNEG = -1e30

F32 = mybir.dt.float32
I32 = mybir.dt.int32


def _ceil_div(a, b):
    return (a + b - 1) // b


# ---------------------------------------------------------------- host prep
def _build_pass_sorted(src_rows, dst, nl, nb, ncores, zrow=0):
    """Degree-sorted dst-partition edge layout.

    Returns:
      perm      [C, nl]   int64: perm[c][j] = original local dst at sorted pos j
      Td        [nb]      int:   unified (max over cores) slots per block
      idx_flat  [C, P, S] int32: kv-table row per (partition, slot), S=sum(Td)
      npad_flat [C, P, nb] f32:  number of pad slots per (partition, block)
    Pad slots carry zrow (caller-provided index of an all-zero table row), so
    they contribute exp(0)=1 to the softmax denominator and 0 to the message;
    the denominator is corrected by subtracting npad.
    """
    core = dst // nl
    dl = (dst % nl).astype(np.int64)

    perms = np.zeros((ncores, nl), np.int64)
    iperms = np.zeros((ncores, nl), np.int64)
    degs = np.zeros((ncores, nb * P), np.int64)
    for c in range(ncores):
        deg = np.bincount(dl[core == c], minlength=nl)
        order = np.argsort(-deg, kind='stable')
        perms[c] = order
        iperms[c][order] = np.arange(nl)
        sdeg = np.zeros(nb * P, np.int64)
        sdeg[:nl] = deg[order]
        degs[c] = sdeg
    Td = degs.reshape(ncores, nb, P).max(2).max(0)
    Td = np.maximum(Td, 1)
    S = int(Td.sum())
    offs = np.zeros(nb + 1, np.int64)
    offs[1:] = np.cumsum(Td)

    idx_flat = np.full((ncores, P, S), zrow, np.int32)
    used = np.zeros((ncores, P, S), bool)
    # per-edge: sorted position -> (block, partition, next free slot)
    for c in range(ncores):
        sel = core == c
        pos = iperms[c][dl[sel]]                     # sorted dst position
        rows = src_rows[sel]
        order = np.argsort(pos, kind='stable')
        pos = pos[order]
        rows = rows[order]
        # slot index within each dst = running count
        bounds = np.flatnonzero(np.diff(pos)) + 1
        starts = np.concatenate([[0], bounds])
        slot = np.arange(len(pos)) - np.repeat(starts, np.diff(
            np.concatenate([starts, [len(pos)]])))
        blk = pos // P
        part = pos % P
        col = offs[blk] + slot
        idx_flat[c, part, col] = rows.astype(np.int32)
        used[c, part, col] = True
    # pad count per (partition, block): subtracted from the softmax denom
    npad_flat = np.zeros((ncores, P, nb), np.float32)
    o = 0
    for b in range(nb):
        npad_flat[:, :, b] = (~used[:, :, o:o + Td[b]]).sum(2)
        o += Td[b]
    # device adds this to z: z - npad + eps. eps=1e-6 (not the reference's
    # 1e-16) so it survives f32 rounding against integer npad; for zero-degree
    # rows z - npad + eps = eps > 0 instead of 0 (no inf/NaN). The epsilon
    # difference perturbs real denominators by <=1e-6 relative.
    npad_flat = np.float32(1e-6) - npad_flat.astype(np.float32)
    return perms, Td, idx_flat, npad_flat, S


def _blockdiag(mats):
    out = np.zeros((D, D), np.float32)
    for h in range(H):
        out[h * DH:(h + 1) * DH, h * DH:(h + 1) * DH] = mats[h]
    return out


def host_prep(inputs, nl):
    nb = _ceil_div(nl, P)
    x_a = np.asarray(inputs['x_author'], np.float32)
    x_p = np.asarray(inputs['x_paper'], np.float32)
    ei_w = np.asarray(inputs['ei_writes'])
    ei_r = np.asarray(inputs['ei_rev'])
    ei_c = np.asarray(inputs['ei_cites'])
    kqv_w = np.asarray(inputs['kqv_w'], np.float32)
    kqv_b = np.asarray(inputs['kqv_b'], np.float32)
    rel_a = np.asarray(inputs['rel_a'], np.float32)
    rel_m = np.asarray(inputs['rel_m'], np.float32)
    p_rel = np.asarray(inputs['p_rel'], np.float32)

    nn = x_a.shape[0]
    ncores = nn // nl

    # ---- papers pass (dst=papers): writes + cites
    src_pap = np.concatenate([ei_w[0], ei_c[0]]).astype(np.int64)
    dst_pap = np.concatenate([ei_w[1], ei_c[1]]).astype(np.int64)
    is_cites = np.concatenate([np.zeros(ei_w.shape[1], bool),
                               np.ones(ei_c.shape[1], bool)])
    src_rev = ei_r[0].astype(np.int64)
    dst_rev = ei_r[1].astype(np.int64)

    # permutations first (needed to map src -> kv table row)
    perm_p, TdP, _, _, SP_ = _build_pass_sorted(
        np.zeros_like(src_pap), dst_pap, nl, nb, ncores)
    perm_a, TdR, _, _, SR_ = _build_pass_sorted(
        np.zeros_like(src_rev), dst_rev, nl, nb, ncores)
    iperm_a = np.zeros((ncores, nl), np.int64)
    iperm_p = np.zeros((ncores, nl), np.int64)
    for c in range(ncores):
        iperm_a[c][perm_a[c]] = np.arange(nl)
        iperm_p[c][perm_p[c]] = np.arange(nl)

    # kv_pap table rows (rank-major, permuted-local, +1 zero row per rank):
    #   rank r: [authors(writes) perm_a | papers(cites) perm_p | zero], 2nl+1
    sc_core = src_pap // nl
    sc_loc = src_pap % nl
    row_pap = np.where(
        is_cites,
        sc_core * (2 * nl + 1) + nl + iperm_p[sc_core, sc_loc],
        sc_core * (2 * nl + 1) + iperm_a[sc_core, sc_loc])
    rc_core = src_rev // nl
    row_rev = rc_core * (nl + 1) + iperm_p[rc_core, src_rev % nl]

    _, TdP, pap_idx, pap_npad, SP_ = _build_pass_sorted(
        row_pap, dst_pap, nl, nb, ncores, zrow=2 * nl)
    _, TdR, rev_idx, rev_npad, SR_ = _build_pass_sorted(
        row_rev, dst_rev, nl, nb, ncores, zrow=nl)

    # ---- folded weights (same as v1)
    rel_cfg = {'writes': (0, 0), 'rev': (1, 1), 'cites': (1, 2)}
    Wkv, Bkv = {}, {}
    for l in range(L):
        for name, (t, r) in rel_cfg.items():
            Ra = _blockdiag(rel_a[l, r] * p_rel[l, r][:, None, None] * SCALE)
            Rm = _blockdiag(rel_m[l, r])
            Wkv[(l, name)] = np.concatenate(
                [kqv_w[l, 0, t] @ Ra, kqv_w[l, 2, t] @ Rm], 1)
            Bkv[(l, name)] = np.concatenate(
                [kqv_b[l, 0, t] @ Ra, kqv_b[l, 2, t] @ Rm], 0)

    Wna = np.stack([np.concatenate([kqv_w[l, 1, 0], Wkv[(l, 'writes')]], 1)
                    for l in range(L)])
    Bna = np.stack([np.broadcast_to(np.concatenate([kqv_b[l, 1, 0],
                                                    Bkv[(l, 'writes')]]), (P, 384))
                    for l in range(L)]).copy()
    Wnp = np.stack([np.concatenate([kqv_w[l, 1, 1], Wkv[(l, 'cites')]], 1)
                    for l in range(L)])
    Bnp = np.stack([np.broadcast_to(np.concatenate([kqv_b[l, 1, 1],
                                                    Bkv[(l, 'cites')]]), (P, 384))
                    for l in range(L)]).copy()
    Wnp2 = np.stack([Wkv[(l, 'rev')] for l in range(L)])
    Bnp2 = np.stack([np.broadcast_to(Bkv[(l, 'rev')], (P, 256))
                     for l in range(L)]).copy()

    out_w = np.asarray(inputs['out_w'], np.float32)
    out_b = np.asarray(inputs['out_b'], np.float32)[..., None]
    skip_rep = np.broadcast_to(np.asarray(inputs['skip'], np.float32)
                               .reshape(1, L * 2), (P, L * 2)).copy()
    proj_w = np.asarray(inputs['proj_w'], np.float32)
    proj_bT = np.asarray(inputs['proj_b'], np.float32)[..., None]
    cls_w = np.asarray(inputs['cls_w'], np.float32)
    cls_b_rep = np.broadcast_to(np.asarray(inputs['cls_b'], np.float32),
                                (P, OUT)).copy()
    ident = np.eye(P, dtype=np.float32)

    shared = dict(Wna=Wna, Bna=Bna, Wnp=Wnp, Bnp=Bnp, Wnp2=Wnp2, Bnp2=Bnp2,
                  Wout=out_w, Bout=out_b, skip_rep=skip_rep,
                  projw=proj_w, projbT=proj_bT, clsw=cls_w, clsb=cls_b_rep,
                  ident=ident)

    in_maps = []
    for c in range(ncores):
        m = dict(shared)
        m['xT_a'] = np.ascontiguousarray(x_a[c * nl + perm_a[c]].T)
        m['xT_p'] = np.ascontiguousarray(x_p[c * nl + perm_p[c]].T)
        m['pap_idx'] = pap_idx[c]
        m['pap_npad'] = pap_npad[c]
        m['rev_idx'] = rev_idx[c]
        m['rev_npad'] = rev_npad[c]
        in_maps.append(m)
    meta = (tuple(int(t) for t in TdP), tuple(int(t) for t in TdR))
    return in_maps, meta, nb, ncores, perm_p


# ---------------------------------------------------------------- device program
def build_program(nl, nb, meta, ncores):
    TdP, TdR = meta
    SP_ = sum(TdP)
    SR_ = sum(TdR)
    nc = bacc.Bacc()
    dp = nc.declare_dram_parameter
    NPAD = nb * P

    xT_a = dp('xT_a', [D, nl], F32, isOutput=False)
    xT_p = dp('xT_p', [D, nl], F32, isOutput=False)
    pap_idx = dp('pap_idx', [P, SP_], I32, isOutput=False)
    pap_npad = dp('pap_npad', [P, nb], F32, isOutput=False)
    rev_idx = dp('rev_idx', [P, SR_], I32, isOutput=False)
    rev_npad = dp('rev_npad', [P, nb], F32, isOutput=False)
    Wna = dp('Wna', [L, D, 384], F32, isOutput=False)
    Bna = dp('Bna', [L, P, 384], F32, isOutput=False)
    Wnp = dp('Wnp', [L, D, 384], F32, isOutput=False)
    Bnp = dp('Bnp', [L, P, 384], F32, isOutput=False)
    Wnp2 = dp('Wnp2', [L, D, 256], F32, isOutput=False)
    Bnp2 = dp('Bnp2', [L, P, 256], F32, isOutput=False)
    Wout = dp('Wout', [L, 2, D, D], F32, isOutput=False)
    Bout = dp('Bout', [L, 2, D, 1], F32, isOutput=False)
    skip_rep = dp('skip_rep', [P, L * 2], F32, isOutput=False)
    projw = dp('projw', [2, D, D], F32, isOutput=False)
    projbT = dp('projbT', [2, D, 1], F32, isOutput=False)
    clsw = dp('clsw', [D, OUT], F32, isOutput=False)
    clsb = dp('clsb', [P, OUT], F32, isOutput=False)
    ident_d = dp('ident', [P, P], F32, isOutput=False)
    logits = dp('logits', [nl, OUT], F32, isOutput=True)

    chunks128 = [(i * P, min(P, nl - i * P)) for i in range(_ceil_div(nl, P))]
    WCH = 512
    chunksw = [(i * WCH, min(WCH, nl - i * WCH)) for i in range(_ceil_div(nl, WCH))]
    TMAXP = max(TdP)
    TMAXR = max(TdR)

    with tile.TileContext(nc) as tc:
        with (
            tc.tile_pool(name='const', bufs=1) as cpool,
            tc.tile_pool(name='wpool', bufs=2) as wpool,
            tc.tile_pool(name='sb', bufs=2) as sb,
            tc.tile_pool(name='edg', bufs=2) as edg,
            tc.tile_pool(name='ps_mm', bufs=2, space='PSUM') as ps_mm,
            tc.tile_pool(name='ps_tps', bufs=2, space='PSUM') as ps_tps,
            tc.tile_pool(name='dram', bufs=1, space='DRAM') as dram,
        ):
            # ---- persistent DRAM scratch
            hT_a = dram.tile([D, nl], F32, tag='hT_a')
            hT_p = dram.tile([D, nl], F32, tag='hT_p')
            hT = {0: hT_a, 1: hT_p}
            q_a = dram.tile([NPAD, D], F32, tag='q_a')
            q_p = dram.tile([NPAD, D], F32, tag='q_p')
            qtab = {0: q_a, 1: q_p}
            aggT_a = dram.tile([D, NPAD], F32, tag='aggT_a')
            aggT_p = dram.tile([D, NPAD], F32, tag='aggT_p')
            aggT = {0: aggT_a, 1: aggT_p}
            kv_pap_in = dram.tile([2 * nl + 1, 256], F32, tag='kv_pap_in')
            kv_rev_in = dram.tile([nl + 1, 256], F32, tag='kv_rev_in')
            kv_pap_l = [dram.tile([(2 * nl + 1) * ncores, 256], F32,
                                  tag=f'kv_pap{i}', name=f'kv_pap{i}',
                                  addr_space='Shared') for i in range(L)]
            kv_rev_l = [dram.tile([(nl + 1) * ncores, 256], F32,
                                  tag=f'kv_rev{i}', name=f'kv_rev{i}',
                                  addr_space='Shared') for i in range(L)]

            # ---- constants
            ident = cpool.tile([P, P], F32)
            nc.sync.dma_start(out=ident[:], in_=ident_d[:, :])
            skip_t = cpool.tile([P, L * 2], F32)
            nc.sync.dma_start(out=skip_t[:], in_=skip_rep[:, :])
            sig_t = cpool.tile([P, L * 2], F32)
            nc.scalar.activation(out=sig_t[:], in_=skip_t[:],
                                 func=mybir.ActivationFunctionType.Sigmoid)

            # ---- initial projection: hT[t] = projw[t].T @ xT + b
            for t, xT in ((0, xT_a), (1, xT_p)):
                pw = wpool.tile([D, D], F32, tag='pw0')
                nc.sync.dma_start(out=pw[:], in_=projw[t, :, :])
                pb = wpool.tile([D, 1], F32, tag='pb')
                nc.sync.dma_start(out=pb[:], in_=projbT[t, :, :])
                for (o, m) in chunksw:
                    xc = sb.tile([D, WCH], F32, tag='xc')
                    nc.sync.dma_start(out=xc[:, :m], in_=xT[:, o:o + m])
                    ps = ps_mm.tile([P, WCH], F32, tag='mm')
                    nc.tensor.matmul(ps[:, :m], lhsT=pw[:], rhs=xc[:, :m],
                                     start=True, stop=True)
                    hc = sb.tile([D, WCH], F32, tag='hc')
                    nc.vector.tensor_scalar_add(out=hc[:, :m], in0=ps[:, :m],
                                                scalar1=pb[:])
                    nc.sync.dma_start(out=hT[t][:, o:o + m], in_=hc[:, :m])

            zero44 = cpool.tile([P, D], F32)
            nc.vector.memset(zero44[:], 0.0)

            for l in range(L):
                # ============ node phase ============
                wna = wpool.tile([D, 384], F32, tag='wna0')
                nc.sync.dma_start(out=wna[:], in_=Wna[l, :, :])
                bna = wpool.tile([P, 384], F32, tag='bna')
                nc.sync.dma_start(out=bna[:], in_=Bna[l, :, :])
                wnp = wpool.tile([D, 384], F32, tag='wnp0')
                nc.sync.dma_start(out=wnp[:], in_=Wnp[l, :, :])
                bnp = wpool.tile([P, 384], F32, tag='bnp')
                nc.sync.dma_start(out=bnp[:], in_=Bnp[l, :, :])
                wnp2 = wpool.tile([D, 256], F32, tag='wnp20')
                nc.sync.dma_start(out=wnp2[:], in_=Wnp2[l, :, :])
                bnp2 = wpool.tile([P, 256], F32, tag='bnp2')
                nc.sync.dma_start(out=bnp2[:], in_=Bnp2[l, :, :])

                for (o, m) in chunksw:
                    hps = sb.tile([D, WCH], F32, tag='hps')
                    nc.sync.dma_start(out=hps[:, :m], in_=hT[1][:, o:o + m])
                    has = sb.tile([D, WCH], F32, tag='has')
                    nc.sync.dma_start(out=has[:, :m], in_=hT[0][:, o:o + m])
                    for g in range(_ceil_div(m, P)):
                        gm = min(P, m - g * P)
                        go = o + g * P
                        hcp = hps[:, g * P:g * P + gm]
                        hca = has[:, g * P:g * P + gm]
                        ps3 = ps_mm.tile([P, 256], F32, tag='mm')
                        nc.tensor.matmul(ps3[:gm, :], lhsT=hcp, rhs=wnp2[:],
                                         start=True, stop=True)
                        qkv3 = sb.tile([P, 256], F32, tag='qkv3')
                        nc.vector.tensor_add(out=qkv3[:gm, :], in0=ps3[:gm, :],
                                             in1=bnp2[:gm, :])
                        nc.sync.dma_start(out=kv_rev_in[go:go + gm, :],
                                          in_=qkv3[:gm, :])

                        ps2 = ps_mm.tile([P, 384], F32, tag='mm')
                        nc.tensor.matmul(ps2[:gm, :], lhsT=hcp, rhs=wnp[:],
                                         start=True, stop=True)
                        qkv2 = sb.tile([P, 384], F32, tag='qkv')
                        nc.vector.tensor_add(out=qkv2[:gm, :], in0=ps2[:gm, :],
                                             in1=bnp[:gm, :])
                        nc.sync.dma_start(out=qtab[1][go:go + gm, :],
                                          in_=qkv2[:gm, :128])
                        nc.sync.dma_start(out=kv_pap_in[nl + go:nl + go + gm, :],
                                          in_=qkv2[:gm, 128:])

                        ps = ps_mm.tile([P, 384], F32, tag='mm')
                        nc.tensor.matmul(ps[:gm, :], lhsT=hca, rhs=wna[:],
                                         start=True, stop=True)
                        qkv = sb.tile([P, 384], F32, tag='qkv')
                        nc.vector.tensor_add(out=qkv[:gm, :], in0=ps[:gm, :],
                                             in1=bna[:gm, :])
                        nc.sync.dma_start(out=qtab[0][go:go + gm, :],
                                          in_=qkv[:gm, :128])
                        nc.sync.dma_start(out=kv_pap_in[go:go + gm, :],
                                          in_=qkv[:gm, 128:])

                if NPAD > nl:
                    for t in (0, 1):
                        nc.sync.dma_start(out=qtab[t][nl:NPAD, :],
                                          in_=zero44[:NPAD - nl, :])
                nc.sync.dma_start(out=kv_pap_in[2 * nl:2 * nl + 1, 0:128],
                                  in_=zero44[0:1, :])
                nc.sync.dma_start(out=kv_pap_in[2 * nl:2 * nl + 1, 128:256],
                                  in_=zero44[0:1, :])
                nc.sync.dma_start(out=kv_rev_in[nl:nl + 1, 0:128],
                                  in_=zero44[0:1, :])
                nc.sync.dma_start(out=kv_rev_in[nl:nl + 1, 128:256],
                                  in_=zero44[0:1, :])

                # ============ allgather kv tables (rev first) ============
                kv_pap, kv_rev = kv_pap_l[l], kv_rev_l[l]
                nc.gpsimd.collective_compute(
                    'AllGather', mybir.AluOpType.bypass,
                    ins=[kv_rev_in[:].opt()], outs=[kv_rev[:].opt()],
                    replica_groups=[list(range(ncores))])
                nc.gpsimd.collective_compute(
                    'AllGather', mybir.AluOpType.bypass,
                    ins=[kv_pap_in[:].opt()], outs=[kv_pap[:].opt()],
                    replica_groups=[list(range(ncores))])

                # ============ edge phase (rev pass first: overlaps pap AG) ====
                for (pi, (t, Td_list, TMAX, tabl, qt, idx_d, npad_d)) in enumerate((
                        (0, TdR, TMAXR, kv_rev, qtab[0], rev_idx, rev_npad),
                        (1, TdP, TMAXP, kv_pap, qtab[1], pap_idx, pap_npad))):
                    S_pass = sum(Td_list)
                    idx_all = edg.tile([P, S_pass], I32, tag=f'idxall{pi}',
                                       bufs=1)
                    nc.sync.dma_start(out=idx_all[:], in_=idx_d[:, :])
                    npad_all = edg.tile([P, nb], F32, tag=f'npadall{pi}',
                                        bufs=1)
                    nc.sync.dma_start(out=npad_all[:], in_=npad_d[:, :])
                    off = 0
                    for b in range(nb):
                        Td = Td_list[b]
                        idx_t = idx_all[:, off:off + Td]
                        npad = npad_all[:, b:b + 1]
                        qb = edg.tile([P, D], F32, tag='qb0')
                        nc.sync.dma_start(out=qb[:], in_=qt[b * P:(b + 1) * P, :])
                        kvg = edg.tile([P, TMAX * 256], F32, tag='kvg')
                        for gi in range(Td):
                            nc.gpsimd.indirect_dma_start(
                                out=kvg[:, gi * 256:(gi + 1) * 256],
                                out_offset=None, in_=tabl[:],
                                in_offset=bass.IndirectOffsetOnAxis(
                                    ap=idx_t[:, gi:gi + 1], axis=0))

                        kvv = kvg[:, :Td * 256].rearrange(
                            'p (t w) -> p t w', w=256)
                        qk = edg.tile([P, TMAX * 128], F32, tag='qk')
                        nc.vector.tensor_mul(
                            out=qk[:, :Td * 128].rearrange(
                                'p (t w) -> p t w', w=128),
                            in0=kvv[:, :, 0:128],
                            in1=qb[:, None, :].to_broadcast([P, Td, 128]))
                        sc = edg.tile([P, TMAX * H], F32, tag='sc')
                        nc.vector.reduce_sum(
                            out=sc[:, :Td * H].rearrange(
                                'p (t h) -> p t h', h=H),
                            in_=qk[:, :Td * 128].rearrange(
                                'p (t h q) -> p t h q', h=H, q=DH),
                            axis=mybir.AxisListType.X)
                        scexp = edg.tile([P, TMAX * H], F32, tag='scexp')
                        nc.scalar.activation(
                            out=scexp[:, :Td * H], in_=sc[:, :Td * H],
                            func=mybir.ActivationFunctionType.Exp)
                        wv = edg.tile([P, TMAX * 128], F32, tag='wv')
                        nc.vector.tensor_mul(
                            out=wv[:, :Td * 128].rearrange(
                                'p (t h q) -> p t h q', h=H, q=DH),
                            in0=kvv[:, :, 128:256].rearrange(
                                'p t (h q) -> p t h q', q=DH),
                            in1=scexp[:, :Td * H].rearrange(
                                'p (t h) -> p t h', h=H)[:, :, :, None]
                                .to_broadcast([P, Td, H, DH]))
                        z = edg.tile([P, H], F32, tag='z')
                        nc.vector.reduce_sum(
                            out=z[:].rearrange('p (o h) -> p o h', o=1),
                            in_=scexp[:, :Td * H].rearrange(
                                'p (t h) -> p h t', h=H)[:, None, :, :],
                            axis=mybir.AxisListType.X)
                        agg0 = edg.tile([P, D], F32, tag='agg0')
                        nc.vector.reduce_sum(
                            out=agg0[:].rearrange('p (o w) -> p o w', o=1),
                            in_=wv[:, :Td * 128].rearrange(
                                'p (t w) -> p w t', w=128)[:, None, :, :],
                            axis=mybir.AxisListType.X)
                        zr = edg.tile([P, H], F32, tag='zr')
                        nc.vector.tensor_scalar_add(out=zr[:], in0=z[:],
                                                    scalar1=npad)
                        zrec = edg.tile([P, H], F32, tag='zrec')
                        nc.vector.reciprocal(out=zrec[:], in_=zr[:])
                        aggd = edg.tile([P, D], F32, tag='aggd')
                        nc.vector.tensor_mul(
                            out=aggd[:].rearrange('p (h q) -> p h q', q=DH),
                            in0=agg0[:].rearrange('p (h q) -> p h q', q=DH),
                            in1=zrec[:, :, None].to_broadcast([P, H, DH]))
                        tps = ps_tps.tile([P, P], F32, tag='tps')
                        nc.tensor.transpose(out=tps[:], in_=aggd[:],
                                            identity=ident[:])
                        aggsb = edg.tile([P, P], F32, tag='aggsb')
                        nc.scalar.activation(
                            out=aggsb[:], in_=tps[:],
                            func=mybir.ActivationFunctionType.Identity)
                        nc.sync.dma_start(out=aggT[t][:, b * P:(b + 1) * P],
                                          in_=aggsb[:])
                        off += Td

                # ============ out phase ============
                for t in (0, 1):
                    wo = wpool.tile([D, D], F32, tag='wo0')
                    nc.sync.dma_start(out=wo[:], in_=Wout[l, t, :, :])
                    bo = wpool.tile([D, 1], F32, tag='bo')
                    nc.sync.dma_start(out=bo[:], in_=Bout[l, t, :, :])
                    for (o, m) in chunksw:
                        ga = sb.tile([D, WCH], F32, tag='ga')
                        nc.sync.dma_start(out=ga[:, :m], in_=aggT[t][:, o:o + m])
                        gag = sb.tile([D, WCH], F32, tag='gag')
                        nc.scalar.activation(out=gag[:, :m], in_=ga[:, :m],
                                             func=mybir.ActivationFunctionType.Gelu)
                        ps = ps_mm.tile([P, WCH], F32, tag='mm')
                        nc.tensor.matmul(ps[:, :m], lhsT=wo[:], rhs=gag[:, :m],
                                         start=True, stop=True)
                        ob = sb.tile([D, WCH], F32, tag='ob')
                        nc.scalar.activation(out=ob[:, :m], in_=ps[:, :m],
                                             func=mybir.ActivationFunctionType.Identity,
                                             bias=bo[:])
                        hld = sb.tile([D, WCH], F32, tag='hld')
                        nc.sync.dma_start(out=hld[:, :m], in_=hT[t][:, o:o + m])
                        # h' = elu(s*o + (1-s)*h) = elu(h + s*(o-h))
                        dif = sb.tile([D, WCH], F32, tag='dif')
                        nc.vector.tensor_sub(out=dif[:, :m], in0=ob[:, :m],
                                             in1=hld[:, :m])
                        sd = sb.tile([D, WCH], F32, tag='sd')
                        nc.vector.tensor_scalar_mul(
                            out=sd[:, :m], in0=dif[:, :m],
                            scalar1=sig_t[:, l * 2 + t:l * 2 + t + 1])
                        hpre = sb.tile([D, WCH], F32, tag='hpre')
                        nc.vector.tensor_add(out=hpre[:, :m], in0=sd[:, :m],
                                             in1=hld[:, :m])
                        neg = sb.tile([D, WCH], F32, tag='neg')
                        nc.vector.tensor_scalar_min(out=neg[:, :m], in0=hpre[:, :m],
                                                    scalar1=0.0)
                        ex = sb.tile([D, WCH], F32, tag='ex')
                        nc.scalar.activation(out=ex[:, :m], in_=neg[:, :m],
                                             func=mybir.ActivationFunctionType.Exp)
                        rl = sb.tile([D, WCH], F32, tag='rl')
                        nc.scalar.activation(out=rl[:, :m], in_=hpre[:, :m],
                                             func=mybir.ActivationFunctionType.Relu)
                        er = sb.tile([D, WCH], F32, tag='er')
                        nc.vector.tensor_add(out=er[:, :m], in0=ex[:, :m],
                                             in1=rl[:, :m])
                        hnew = sb.tile([D, WCH], F32, tag='hnew')
                        nc.vector.tensor_scalar_add(out=hnew[:, :m], in0=er[:, :m],
                                                    scalar1=-1.0)
                        nc.sync.dma_start(out=hT[t][:, o:o + m], in_=hnew[:, :m])

            # ============ classifier ============
            cw = cpool.tile([D, OUT], F32)
            nc.sync.dma_start(out=cw[:], in_=clsw[:, :])
            cb = cpool.tile([P, OUT], F32)
            nc.sync.dma_start(out=cb[:], in_=clsb[:, :])
            for (o, m) in chunks128:
                hc = sb.tile([D, P], F32, tag='hca')
                nc.sync.dma_start(out=hc[:, :m], in_=hT[1][:, o:o + m])
                ps = ps_mm.tile([P, OUT], F32, tag='mm')
                nc.tensor.matmul(ps[:m, :], lhsT=hc[:, :m], rhs=cw[:],
                                 start=True, stop=True)
                lg = sb.tile([P, OUT], F32, tag='lg')
                nc.vector.tensor_add(out=lg[:m, :], in0=ps[:m, :], in1=cb[:m, :])
                nc.sync.dma_start(out=logits[o:o + m, :], in_=lg[:m, :])

    nc.finalize()
    return nc


# ---------------------------------------------------------------- entry point
_CACHE = {}


def kernel(**inputs):
    nn = np.asarray(inputs['x_author']).shape[0]
    nl = nn // C
    in_maps, meta, nb, ncores, perm_p = host_prep(inputs, nl)
    key = (nl, nb, meta, ncores)
    if key not in _CACHE:
        _CACHE[key] = build_program(nl, nb, meta, ncores)
    nc = _CACHE[key]
    res = run_bass_kernel_spmd(nc, in_maps, list(range(ncores)))
    out = np.zeros((nn, OUT), np.float32)
    for c in range(ncores):
        out[c * nl + perm_p[c]] = res.results[c]['logits']
    return out


if __name__ == '__main__':
    pass
